# revision 1
# baseline (speedup 1.0000x reference)
"""GQA attention block (dense_transformer) on 8 Trainium2 NeuronCores.

Sharding: tensor-parallel over heads (4 groups) x data-parallel over batch (2).
Core c handles batch b = c // 4 and head group g = c % 4 (8 query heads, 2 KV
heads). Wq/Wk/Wv are sharded column-wise (output dims), Wo row-wise; the
row-parallel output partials are summed on the host (the "all-reduce").

Per-core device kernel (all matmuls in float32r = FP22-truncated fp32, full
PE rate at N=512):
  phase 1: ONE streaming pass over X^T computes Q^T, K^T, V^T together
           (6 concurrent PSUM accumulators); V^T -> V via PE transpose,
           augmented with a ones column per KV head so the context matmul
           also emits the softmax denominator row.
  per q-tile s (512 queries): a flat software pipeline over (head, key-pair)
           steps: ACT does exp on double-width [128,1024] tiles (two key
           tiles at once) while PE runs scores of step i and ctx of step
           i-1, plus one out-proj(s-1) filler matmul every other step.
           Each head's normalization (reciprocal of the denominator row,
           ones-matmul partition broadcast, DVE multiply) is deferred into
           the next head's stream, off the PE critical path.

Head-block permutation [0,4,1,5,2,6,3,7] applied to Wq rows / Wo.T rows on
the host makes every PSUM evacuation partition-aligned (kv0 heads at
partitions 0-63, kv1 heads at 64-127).

HW-validated constraints honored here:
 - producers of f32r-matmul inputs write f32r dtype (DMA via bitcast).
 - memset cannot write f32r (copy from an fp32 ones column instead).
 - PSUM accumulation groups never mix base partitions.
 - each matmul's PSUM output stays within one 2KB bank.
"""

import numpy as np

H, HKV, HD, D = 32, 8, 64, 2048
B, S = 2, 2048
N_CORES = 8
TPG = 4              # tensor-parallel groups (head groups)
QH = H // TPG        # 8 query heads per core
KVH = HKV // TPG     # 2 kv heads per core
SCALE = 1.0 / 8.0    # 1/sqrt(HD)
PERM = [0, 4, 1, 5, 2, 6, 3, 7]   # head-block permutation

NK = D // 128        # 16 contraction chunks for projections
NST = S // 512       # 4 token tiles
KT = S // 128        # 16 key tiles
KP = KT // 2         # 8 key-tile PAIRS (exp granularity 1024)

_nc_cache = {}


def _build_nc(loop_n=None):
    import concourse.bacc as bacc
    import concourse.tile as tile
    import concourse.mybir as mybir
    from concourse.masks import make_identity

    fp32 = mybir.dt.float32
    f32r = mybir.dt.float32r
    AF = mybir.ActivationFunctionType

    nc = bacc.Bacc(None, target_bir_lowering=False, debug=False)

    xT = nc.dram_tensor("xT", [D, S], fp32, kind="ExternalInput")
    wqT = nc.dram_tensor("wqT", [D, QH * HD], fp32, kind="ExternalInput")
    wkT = nc.dram_tensor("wkT", [D, KVH * HD], fp32, kind="ExternalInput")
    wvT = nc.dram_tensor("wvT", [D, KVH * HD], fp32, kind="ExternalInput")
    woT = nc.dram_tensor("woT", [QH * HD, D], fp32, kind="ExternalInput")
    out = nc.dram_tensor("out", [S, D], fp32, kind="ExternalOutput")

    with tile.TileContext(nc) as tc:
        with (
            tc.tile_pool(name="persist", bufs=1) as persist,
            tc.tile_pool(name="xs", bufs=3) as xs,
            tc.tile_pool(name="qpool", bufs=4) as qpool,
            tc.tile_pool(name="cpool", bufs=2) as cpool,
            tc.tile_pool(name="probs", bufs=3) as probs,
            tc.tile_pool(name="ostage", bufs=4) as ostage,
            tc.tile_pool(name="smalls", bufs=2) as smalls,
            # one PSUM pool, 8 banks total:
            #   tag "sc": 2 x [128,1024] (4 banks) - scores pairs; also
            #             phase-1 K/V accumulators and V transposes
            #   tag "cx": 2 x [128,512]  (2 banks) - ctx accumulators (+Q0/Q1)
            #   tag "op": 2 x [128,512]  (2 banks) - out-proj blocks, ps_b,
            #             phase-1 Q2/Q3
            tc.tile_pool(name="ps", bufs=2, space="PSUM") as ps,
        ):
            from contextlib import nullcontext
            import concourse.mybir as _mb
            _loop_ctx = (tc.For_i(0, loop_n, 1, hint_engines=(
                _mb.EngineType.PE, _mb.EngineType.Activation,
                _mb.EngineType.DVE, _mb.EngineType.SP,
                _mb.EngineType.Pool)) if loop_n else nullcontext())
            with _loop_ctx:
                # ---- weights (wq per-chunk just-in-time; wo deferred) ----
                wk_sb = persist.tile([128, NK, KVH * HD], f32r)
                nc.scalar.dma_start(
                    out=wk_sb, in_=wkT.bitcast(f32r).rearrange("(k p) m -> p k m", p=128))
                wv_sb = persist.tile([128, NK, KVH * HD], f32r)
                nc.scalar.dma_start(
                    out=wv_sb, in_=wvT.bitcast(f32r).rearrange("(k p) m -> p k m", p=128))
                wq_sb = persist.tile([128, NK, QH * HD], f32r)
                wqT_r = wqT.bitcast(f32r).rearrange("(k p) m -> p k m", p=128)
                for k in range(NK):
                    nc.scalar.dma_start(out=wq_sb[:, k, :], in_=wqT_r[:, k, :])
                wo_sb = persist.tile([128, 4, D], f32r)

                ones_col = persist.tile([128, 1], fp32)
                nc.vector.memset(ones_col, 1.0)
                ident = persist.tile([128, 128], fp32)
                make_identity(nc, ident)
                ones_row = persist.tile([1, 64], f32r)
                nc.vector.tensor_copy(ones_row, ones_col[0:1, 0:1].to_broadcast([1, 64]))

                kT_sb = persist.tile([128, S], f32r)        # [hd(2 kv), keys]
                v_aug = persist.tile([128, KT, 130], f32r)  # [keys(128/t), t, 65*2]

                # vT shares the cpool "cT" slots (released before ctx tile #2)
                vT = cpool.tile([128, S], fp32, tag="cT", name="vT")

                qT_tiles = {}
                ctx_tiles = {}

                # ---- phase 1: Q^T/K^T/V^T in one streaming pass over X^T ----
                for st in range(NST):
                    # K and V share one double-wide "sc" tile (halves = separate
                    # banks, separate accumulation groups), so the second "sc"
                    # slot stays free for this tile's V transposes.
                    ps_kv = ps.tile([128, 1024], fp32, tag="sc", name="ps_kv")
                    ps_q = [
                        ps.tile([128, 512], fp32, tag="cx", name="ps_q0"),
                        ps.tile([128, 512], fp32, tag="cx", name="ps_q1"),
                        ps.tile([128, 512], fp32, tag="op", name="ps_q2"),
                        ps.tile([128, 512], fp32, tag="op", name="ps_q3"),
                    ]
                    for k in range(NK):
                        xa = xs.tile([128, 512], f32r, tag="xs", name="xa")
                        nc.sync.dma_start(
                            out=xa,
                            in_=xT[k * 128:(k + 1) * 128, st * 512:(st + 1) * 512].bitcast(f32r))
                        st_, sp_ = (k == 0), (k == NK - 1)
                        nc.tensor.matmul(ps_kv[:, 0:512], wk_sb[:, k, :], xa,
                                         start=st_, stop=sp_)
                        nc.tensor.matmul(ps_kv[:, 512:1024], wv_sb[:, k, :], xa,
                                         start=st_, stop=sp_)
                        for m in range(4):
                            nc.tensor.matmul(ps_q[m], wq_sb[:, k, m * 128:(m + 1) * 128],
                                             xa, start=st_, stop=sp_)
                    nc.vector.tensor_copy(kT_sb[:, st * 512:(st + 1) * 512],
                                          ps_kv[:, 0:512])
                    nc.vector.tensor_copy(vT[:, st * 512:(st + 1) * 512],
                                          ps_kv[:, 512:1024])
                    qT_s = qpool.tile([128, 4, 512], f32r, tag="qT", name="qT_s")
                    for m in range(4):
                        nc.vector.tensor_copy(qT_s[:, m, :], ps_q[m])
                    qT_tiles[st] = qT_s
                    for tt in range(4 * st, 4 * st + 4):
                        ps_t = ps.tile([128, 128], fp32, tag="sc", name="ps_t")
                        nc.tensor.transpose(ps_t, vT[:, tt * 128:(tt + 1) * 128], ident)
                        nc.vector.tensor_copy(v_aug[:, tt, 0:64], ps_t[:, 0:64])
                        nc.vector.tensor_copy(v_aug[:, tt, 65:129], ps_t[:, 64:128])
                        nc.vector.tensor_copy(v_aug[:, tt, 64:65], ones_col)
                        nc.vector.tensor_copy(v_aug[:, tt, 129:130], ones_col)
                    if st == 0:
                        # 4 MB of Wo, first needed by out-proj(0) during attn(1)
                        nc.scalar.dma_start(
                            out=wo_sb,
                            in_=woT.bitcast(f32r).rearrange("(c p) n -> p c n", p=128))

                def outproj_filler_gen(s):
                    """Yield 64 closures emitting the output projection of tile s
                    one matmul at a time; each (i, n) block accumulates its 4
                    c-chunks into a single PSUM and evacuates inline."""
                    ctxT_s = ctx_tiles[s]
                    state = {}
                    for i in range(4):
                        for n in range(4):
                            for c in range(4):
                                def emit(i=i, n=n, c=c):
                                    if c == 0:
                                        state["ps_o"] = ps.tile(
                                            [128, 512], fp32, tag="op", name="ps_o")
                                    nc.tensor.matmul(
                                        state["ps_o"],
                                        ctxT_s[:, c, i * 128:(i + 1) * 128],
                                        wo_sb[:, c, n * 512:(n + 1) * 512],
                                        start=(c == 0), stop=(c == 3))
                                    if c == 3:
                                        ob = ostage.tile([128, 512], fp32,
                                                         tag="ob", name="ob")
                                        nc.vector.tensor_copy(ob, state["ps_o"])
                                        nc.sync.dma_start(
                                            out=out[s * 512 + i * 128:
                                                    s * 512 + (i + 1) * 128,
                                                    n * 512:(n + 1) * 512],
                                            in_=ob)
                                yield emit

                # ---- attention: flat (head, key-pair) pipeline per q-tile ----
                pending_norm = [None]

                def make_norm(ps_c, p_, half, s):
                    def norm():
                        rec_r = smalls.tile([1, 512], f32r, tag="recr", name="rec_r")
                        with nc.allow_low_precision(
                                "f32r is fp32-width; PE rounds to fp22 on read"):
                            nc.vector.reciprocal(rec_r, ps_c[64:65, :])
                        ps_b = ps.tile([64, 512], fp32, tag="op", name="ps_b")
                        nc.tensor.matmul(ps_b, ones_row, rec_r, start=True, stop=True)
                        rbc = smalls.tile([64, 512], fp32, tag="rbc", name="rbc")
                        nc.vector.tensor_copy(rbc, ps_b)
                        ctxT_s = ctx_tiles[s]
                        if half == 0:
                            nc.vector.tensor_mul(ctxT_s[0:64, p_, :], ps_c[0:64, :], rbc)
                        else:
                            ctmp = smalls.tile([64, 512], f32r, tag="ctmp", name="ctmp")
                            nc.vector.tensor_mul(ctmp, ps_c[0:64, :], rbc)
                            nc.sync.dma_start(out=ctxT_s[64:128, p_, :], in_=ctmp)
                    return norm

                for s in range(NST):
                    ctx_tiles[s] = cpool.tile([128, 4, 512], f32r, tag="cT",
                                              name="ctxT_s")
                    fillers = iter(outproj_filler_gen(s - 1)) if s > 0 else iter(())
                    qT_s = qT_tiles[s]

                    # flat step list: step = (head, key-pair kp)
                    steps = [(h, kp) for h in range(QH) for kp in range(KP)]
                    n_steps = len(steps)          # 64
                    sc_tiles = [None] * n_steps
                    pb_tiles = [None] * n_steps
                    ctx_ps = {}

                    def emit_scores(i):
                        h, kp = steps[i]
                        half = h // 4
                        lo, hi = half * 64, half * 64 + 64
                        ps_s = ps.tile([128, 1024], fp32, tag="sc", name="ps_s")
                        for u in range(2):
                            t = 2 * kp + u
                            nc.tensor.matmul(
                                ps_s[:, u * 512:(u + 1) * 512],
                                kT_sb[lo:hi, t * 128:(t + 1) * 128],
                                qT_s[lo:hi, h % 4, :], start=True, stop=True)
                        sc_tiles[i] = ps_s

                    def emit_exp(i):
                        pb = probs.tile([128, 1024], f32r, tag="pb", name="pb")
                        nc.scalar.activation(pb, sc_tiles[i], AF.Exp, scale=SCALE)
                        pb_tiles[i] = pb

                    def emit_ctx(i):
                        h, kp = steps[i]
                        half = h // 4
                        if kp == 0:
                            ctx_ps[h] = ps.tile([128, 512], fp32, tag="cx",
                                                name="ps_c")
                        pc = ctx_ps[h]
                        for u in range(2):
                            t = 2 * kp + u
                            nc.tensor.matmul(
                                pc[0:65, :], v_aug[:, t, half * 65:half * 65 + 65],
                                pb_tiles[i][:, u * 512:(u + 1) * 512],
                                start=(t == 0), stop=(t == KT - 1))
                        if kp == KP - 1:
                            # head finished: defer normalization into next head
                            if pending_norm[0] is not None:
                                pending_norm[0]()
                            pending_norm[0] = make_norm(pc, h % 4, half, s)

                    emit_scores(0)
                    for i in range(1, n_steps):
                        emit_exp(i - 1)
                        emit_scores(i)
                        emit_ctx(i - 1)
                        f = next(fillers, None)
                        if f is not None:
                            f()
                    emit_exp(n_steps - 1)
                    emit_ctx(n_steps - 1)
                    for f in fillers:
                        f()
                    # flush the last head's norm before next tile reuses slots
                    if pending_norm[0] is not None:
                        pending_norm[0]()
                        pending_norm[0] = None

                # epilogue: output projection of the last tile
                for f in outproj_filler_gen(NST - 1):
                    f()

    nc.finalize()
    return nc


def _get_nc(loop_n=None):
    key = ("nc", loop_n)
    if key not in _nc_cache:
        _nc_cache[key] = _build_nc(loop_n)
    return _nc_cache[key]


def _prep_core_inputs(hidden_states, Wq, Wk, Wv, Wo):
    perm = np.array(PERM)
    in_maps = []
    xTb = [np.ascontiguousarray(hidden_states[b].T) for b in range(B)]
    for c in range(N_CORES):
        b, g = c // TPG, c % TPG
        wq_g = Wq[g * QH * HD:(g + 1) * QH * HD]            # [512, D]
        wq_gp = wq_g.reshape(QH, HD, D)[perm].reshape(QH * HD, D)
        wo_g = Wo[:, g * QH * HD:(g + 1) * QH * HD]          # [D, 512]
        wo_gtp = wo_g.T.reshape(QH, HD, D)[perm].reshape(QH * HD, D)
        in_maps.append({
            "xT": xTb[b],
            "wqT": np.ascontiguousarray(wq_gp.T),
            "wkT": np.ascontiguousarray(Wk[g * KVH * HD:(g + 1) * KVH * HD].T),
            "wvT": np.ascontiguousarray(Wv[g * KVH * HD:(g + 1) * KVH * HD].T),
            "woT": np.ascontiguousarray(wo_gtp),
        })
    return in_maps


def _numpy_reference(hidden_states, attention_mask, Wq, Wk, Wv, Wo):
    """Host fallback, only used if a nonzero attention mask ever shows up."""
    groups = H // HKV
    q = (hidden_states @ Wq.T).reshape(B, S, H, HD).transpose(0, 2, 1, 3)
    k = (hidden_states @ Wk.T).reshape(B, S, HKV, HD).transpose(0, 2, 1, 3)
    v = (hidden_states @ Wv.T).reshape(B, S, HKV, HD).transpose(0, 2, 1, 3)
    k = np.repeat(k, groups, axis=1)
    v = np.repeat(v, groups, axis=1)
    scores = np.einsum('bhqd,bhkd->bhqk', q, k) * np.float32(SCALE)
    scores = scores + attention_mask
    scores = scores - scores.max(axis=-1, keepdims=True)
    e = np.exp(scores)
    probs = e / e.sum(axis=-1, keepdims=True)
    ctx = np.einsum('bhqk,bhkd->bhqd', probs, v)
    ctx = ctx.transpose(0, 2, 1, 3).reshape(B, S, H * HD)
    return (ctx @ Wo.T).astype(np.float32)


def run_sharded(hidden_states, attention_mask, Wq, Wk, Wv, Wo, **bass_kwargs):
    """Run the SPMD kernel; returns (output [B,S,D], BassKernelResults)."""
    from concourse.bass_utils import run_bass_kernel_spmd

    nc = _get_nc()
    in_maps = _prep_core_inputs(hidden_states, Wq, Wk, Wv, Wo)
    res = run_bass_kernel_spmd(nc, in_maps, list(range(N_CORES)), **bass_kwargs)
    outs = []
    for b in range(B):
        acc = res.results[b * TPG]["out"].astype(np.float32)
        for g in range(1, TPG):
            acc = acc + res.results[b * TPG + g]["out"]
        outs.append(acc)
    return np.stack(outs, axis=0), res


def kernel(hidden_states, attention_mask, Wq, Wk, Wv, Wo):
    if attention_mask is not None and np.any(attention_mask):
        return _numpy_reference(hidden_states, attention_mask, Wq, Wk, Wv, Wo)
    out, _ = run_sharded(hidden_states, attention_mask, Wq, Wk, Wv, Wo)
    return out

